# revision 1
# baseline (speedup 1.0000x reference)
"""Trainium2 Bass kernel for nn_ChimeraNet (encoder -> 10-step Euler RNN -> LN -> readout).

Data-parallel over 8 NeuronCores: each core gets 1024 rows of the batch and a
replicated set of (host-prefolded) weights.

Math (per core, R=1024 rows, D=1024):
    drive_in = x @ W_c + bias          with W_c = W_enc.T @ W_in (host-folded)
    h_{t+1}  = 0.8 h_t + 0.2 tanh(h_t @ W_res + drive_in),  h_0 = 0, 10 steps
    out      = inv*(h @ W2.T) + (-mu*inv)*w1 + b2           (LayerNorm folded into readout)
  where mu/var are LayerNorm stats over D, inv = rsqrt(var+eps),
  W2 = W_out * ln_g,  w1 = W2 @ 1,  b2 = W_out @ ln_b + b_out.

The integration state is stored scaled, u_t = h_t / 0.2, with 0.2 folded into
W_res and the readout weights on the host.  The recurrence becomes
    u_{t+1} = 0.8*u_t + tanh(u_t @ (0.2 W_res) + drive_in)
so the state update is a single fused scalar_tensor_tensor DVE op per tile.

On-chip layout: the state is kept TRANSPOSED (u.T, D on partitions, rows on the
free dim) so every matmul is weight-stationary (lhsT = natural weight block) and
no per-step transposes are needed.  Matmuls run as float32r (full PE rate at
N=512); accumulation is fp32 in PSUM.  LayerNorm stats over D (the partition
dim) are computed on PE: the sum via an extra ones-column in the readout lhsT,
the sum of squares via ones-vector matmuls on squared tiles.
"""

import os
import sys

import numpy as np

try:
    import concourse.bass as bass  # noqa: F401
except ImportError:  # pragma: no cover - fresh grading env without PYTHONPATH
    for p in ("/root/.axon_site", "/root/.axon_site/_ro/trn_rl_repo",
              "/root/.axon_site/_ro/pypackages", "/opt/trn_rl_repo"):
        if os.path.isdir(p) and p not in sys.path:
            sys.path.append(p)
    import concourse.bass as bass

from contextlib import ExitStack

import concourse.tile as tile
from concourse import bacc, bass_utils, mybir
from concourse.masks import make_identity

N_CORES = 8
B = 8192
R = B // N_CORES        # rows per core
D = 1024                # latent dim
KX = 784                # encoder input dim
DT_STEP = 0.2
STEPS = 10
EPS = 1e-5

F32 = mybir.dt.float32
F32R = mybir.dt.float32r
AF = mybir.ActivationFunctionType
ALU = mybir.AluOpType

KD = D // 128           # 8 k/m tiles over D
NS = R // 512           # 2 moving-dim slices of 512
KXT = [128] * 6 + [16]  # 784 = 6*128 + 16
NWARM = 6              # PE warmup matmuls (HAM un-throttle during DMA wait)


def _build_program():
    nc = bacc.Bacc("TRN2", target_bir_lowering=False, debug=False)

    x = nc.dram_tensor("x", [R, KX], F32, kind="ExternalInput").ap()
    w_c = nc.dram_tensor("w_c", [KX, D], F32, kind="ExternalInput").ap()
    w_res = nc.dram_tensor("w_res", [D, D], F32, kind="ExternalInput").ap()
    bias = nc.dram_tensor("bias", [D], F32, kind="ExternalInput").ap()
    w2a = nc.dram_tensor("w2a", [D, 11], F32, kind="ExternalInput").ap()
    w1 = nc.dram_tensor("w1", [10], F32, kind="ExternalInput").ap()
    b2 = nc.dram_tensor("b2", [10], F32, kind="ExternalInput").ap()
    out = nc.dram_tensor("out", [R, 10], F32, kind="ExternalOutput").ap()

    with tile.TileContext(nc) as tc, ExitStack() as ctx:
        state = ctx.enter_context(tc.tile_pool(name="state", bufs=1))
        consts = ctx.enter_context(tc.tile_pool(name="consts", bufs=1))
        wres_pool = ctx.enter_context(tc.tile_pool(name="wres", bufs=1))

        # persistent SBUF state: double-buffered transposed u, plus drive_in
        g = [[state.tile([128, R], F32R, name=f"g{b}_{k}", tag=f"g{b}_{k}") for k in range(KD)]
             for b in range(2)]
        drive = [state.tile([128, R], F32, name=f"dr{k}", tag=f"dr{k}") for k in range(KD)]
        wres_sb = [wres_pool.tile([128, D], F32R, name=f"wr{k}", tag=f"wr{k}") for k in range(KD)]

        with ExitStack() as mmctx:
            psum = mmctx.enter_context(
                tc.tile_pool(name="mm", bufs=4, space="PSUM"))
            if True:
                # PE warmup: dependency-free fp32 matmuls starting at t~0 pull
                # the HAM clock gate to 8/8 while the input DMAs are in flight
                # (transpose-mode does not count as PE activity for HAM).
                warm_src = consts.tile([128, 256], F32)
                nc.vector.memset(warm_src, 0.01)
                warm_sb = consts.tile([128, 1], F32)
                for w in range(NWARM):
                    wp = psum.tile([128, 512], F32, name=f"warm{w}", tag="mm")
                    nc.tensor.matmul(wp[:, :256], lhsT=warm_src[:, :128], rhs=warm_src,
                                     start=True, stop=True)
                    if w == NWARM - 1:
                        nc.vector.tensor_copy(warm_sb, wp[:, :1])  # keep-alive

                ident = consts.tile([128, 128], F32)
                make_identity(nc, ident)
                bias_sb = consts.tile([128, KD], F32)
                nc.gpsimd.dma_start(out=bias_sb, in_=bias.rearrange("(m p) -> p m", p=128))

                # ------------ encoder: x -> x.T, drive_in = x @ W_c + bias ----
                with ExitStack() as enc:
                    xn_pool = enc.enter_context(tc.tile_pool(name="xn", bufs=4))
                    xt_pool = enc.enter_context(tc.tile_pool(name="xt", bufs=1))
                    wc_pool = enc.enter_context(tc.tile_pool(name="wc", bufs=1))
                    etp = enc.enter_context(
                        tc.tile_pool(name="etp", bufs=4, space="PSUM"))

                    # x tiles on the sync (HWDGE) queue, first in program order
                    # so the transposes can start as early as possible; weights
                    # go on the gpsimd (SWDGE) queue so they don't block x.
                    xt_big = xt_pool.tile([128, len(KXT), R], F32R, name="xt_big")
                    wc_sb = [wc_pool.tile([128, D], F32R, name=f"wc{k}", tag=f"wc{k}")
                             for k in range(len(KXT))]
                    for k, kw in enumerate(KXT):
                        nc.scalar.dma_start(out=wc_sb[k][:kw, :],
                                            in_=w_c[k * 128:k * 128 + kw, :].bitcast(F32R))
                    def transpose_rt(rt):
                        xn = xn_pool.tile([128, KX], F32, name=f"xn{rt}", tag="xn")
                        nc.sync.dma_start(out=xn, in_=x[rt * 128:(rt + 1) * 128, :])
                        rsl = slice(rt * 128, (rt + 1) * 128)
                        # dependency-free matmul BEFORE the transposes: it runs
                        # during this tile's DMA wait (PE is in-order), keeping
                        # the HAM window busy so the clock gate stays at 8/8
                        wp0 = psum.tile([128, 512], F32, name=f"wmh{rt}", tag="mm")
                        nc.tensor.matmul(wp0[:, :256], lhsT=warm_src[:, :128],
                                         rhs=warm_src, start=True, stop=True)
                        # transpose two 128-blocks into one psum tile, then one
                        # strided copy moves both into the x.T tensor
                        for kp in range(3):
                            pt = etp.tile([128, 256], F32, name=f"pt{rt}_{kp}", tag="tp")
                            for h in range(2):
                                k = 2 * kp + h
                                nc.tensor.transpose(pt[:, h * 128:(h + 1) * 128],
                                                    xn[:, k * 128:(k + 1) * 128], ident)
                            src = pt.rearrange("p (two c) -> p two c", two=2)
                            dst = xt_big[:, 2 * kp:2 * kp + 2, rsl]
                            if kp % 2 == 0:
                                nc.scalar.copy(dst, src)
                            else:
                                nc.vector.tensor_copy(dst, src)
                        pt = etp.tile([128, 256], F32, name=f"pt{rt}_3", tag="tp")
                        nc.tensor.transpose(pt[:16, :128], xn[:, 768:784], ident)
                        nc.vector.tensor_copy(xt_big[:16, 6, rsl], pt[:16, :128])

                    def encoder_mms(n):
                        sl = slice(n * 512, (n + 1) * 512)
                        for m in range(KD):
                            ps = psum.tile([128, 512], F32, name=f"eps{n}_{m}", tag="mm")
                            for k, kw in enumerate(KXT):
                                nc.tensor.matmul(
                                    ps,
                                    lhsT=wc_sb[k][:kw, m * 128:(m + 1) * 128],
                                    rhs=xt_big[:kw, k, sl],
                                    start=(k == 0), stop=(k == len(KXT) - 1))
                            nc.scalar.activation(drive[m][:, sl], ps, AF.Identity,
                                                 bias=bias_sb[:, m:m + 1], scale=1.0)

                    # interleave: the n=0 encoder matmuls run (and keep the PE
                    # clock warm) while rows 4-7 are still DMA-ing in
                    for rt in range(4):
                        transpose_rt(rt)
                    encoder_mms(0)
                    for rt in range(4, 8):
                        transpose_rt(rt)
                    encoder_mms(1)

                # W_res arrives on the gpsimd queue while the encoder runs.
                for k in range(KD):
                    nc.gpsimd.dma_start(out=wres_sb[k],
                                        in_=w_res[k * 128:(k + 1) * 128, :].bitcast(F32R))

                # ------------ Euler integration loop (u-state) ----------------
                tmp = ctx.enter_context(tc.tile_pool(name="tmp", bufs=3))

                # step 0: u1 = tanh(drive_in)
                for n in range(NS):
                    sl = slice(n * 512, (n + 1) * 512)
                    for m in range(KD):
                        nc.scalar.activation(g[0][m][:, sl], drive[m][:, sl], AF.Tanh)

                sqp = ctx.enter_context(tc.tile_pool(name="sq", bufs=1))
                sq_tiles = [[sqp.tile([128, 512], F32R, name=f"sq{n}_{k}", tag=f"sq{n}_{k}")
                             for k in range(KD)] for n in range(NS)]

                for s in range(1, STEPS):
                    cur, nxt = g[(s + 1) % 2], g[s % 2]
                    for n in range(NS):
                        sl = slice(n * 512, (n + 1) * 512)
                        for m in range(KD):
                            ps = psum.tile([128, 512], F32, name=f"ps{s}_{n}_{m}", tag="mm")
                            for k in range(KD):
                                nc.tensor.matmul(
                                    ps,
                                    lhsT=wres_sb[k][:, m * 128:(m + 1) * 128],
                                    rhs=cur[k][:, sl],
                                    start=(k == 0), stop=(k == KD - 1))
                            d = tmp.tile([128, 512], F32, name=f"d{s}_{n}_{m}", tag="d")
                            nc.vector.tensor_add(d, ps, drive[m][:, sl])
                            t = tmp.tile([128, 512], F32, name=f"t{s}_{n}_{m}", tag="t")
                            nc.scalar.activation(t, d, AF.Tanh)
                            # u' = 0.8*u + t  (single fused op)
                            nc.vector.scalar_tensor_tensor(
                                nxt[m][:, sl], in0=cur[m][:, sl], scalar=1.0 - DT_STEP,
                                in1=t, op0=ALU.mult, op1=ALU.add)
                            if s == STEPS - 1:
                                # square for the LN variance, overlapped here so
                                # the tail matmuls don't wait on a serial chain
                                nc.scalar.activation(sq_tiles[n][m], nxt[m][:, sl],
                                                     AF.Square)

                gfin = g[(STEPS - 1) % 2]

                # ------------ tail: LN stats + readout (matmul part) ----------
                tail = ctx.enter_context(tc.tile_pool(name="tail", bufs=1))

                ones_f32 = tail.tile([128, 1], F32)
                nc.vector.memset(ones_f32, 1.0)
                ones_sb = tail.tile([128, 1], F32R)
                nc.scalar.copy(ones_sb, ones_f32)
                eps_sb = tail.tile([128, 1], F32)
                nc.vector.memset(eps_sb, EPS)
                # w2a = [0.2*W2.T | ones] : readout weights + S1 column
                w2a_sb = tail.tile([128, KD, 11], F32R)
                nc.gpsimd.dma_start(out=w2a_sb,
                                    in_=w2a.rearrange("(k p) o -> p k o", p=128).bitcast(F32R))
                w1_bc = tail.tile([128, 10], F32)
                nc.gpsimd.dma_start(out=w1_bc, in_=bass.AP(tensor=w1.tensor, offset=w1.offset,
                                                           ap=[[0, 128]] + list(w1.ap)))
                b2_bc = tail.tile([128, 10], F32)
                nc.gpsimd.dma_start(out=b2_bc, in_=bass.AP(tensor=b2.tensor, offset=b2.offset,
                                                           ap=[[0, 128]] + list(b2.ap)))

                s2_sb = tail.tile([1, R], F32)
                y_sb = tail.tile([11, R], F32)

                # per-n readout matmuls, with the per-row-tile stat/combine
                # chains interleaved so the n=0 half finishes while n=1 runs.
                # y_sb rows 0-9 = 0.2*(W2 @ u.T) = W2 @ h.T;  row 10 = sum_D u.
                tp2ctx = ExitStack()
                tp2 = tp2ctx.enter_context(
                    tc.tile_pool(name="tp2", bufs=4, space="PSUM"))
                for n in range(NS):
                    sl = slice(n * 512, (n + 1) * 512)
                    yp = psum.tile([11, 512], F32, name=f"yp{n}", tag="mm")
                    for k in range(KD):
                        nc.tensor.matmul(yp, lhsT=w2a_sb[:, k, :],
                                         rhs=gfin[k][:, sl],
                                         start=(k == 0), stop=(k == KD - 1))
                    nc.scalar.copy(y_sb[:, sl], yp)
                    s2 = psum.tile([1, 512], F32, name=f"s2p{n}", tag="mm")
                    for k in range(KD):
                        nc.tensor.matmul(s2, lhsT=ones_sb, rhs=sq_tiles[n][k],
                                         start=(k == 0), stop=(k == KD - 1))
                    nc.scalar.copy(s2_sb[:, sl], s2)

                    for rt in range(n * 4, (n + 1) * 4):
                        sl = slice(rt * 128, (rt + 1) * 128)
                        yn = tp2.tile([128, 11], F32, name=f"yn{rt}", tag="st")
                        nc.tensor.transpose(yn, y_sb[:, sl], ident[:11, :11])
                        p2 = tp2.tile([128, 1], F32, name=f"p2_{rt}", tag="st")
                        nc.tensor.transpose(p2, s2_sb[:, sl], ident[:1, :1])
                        mu_n = tail.tile([128, 1], F32, name=f"mu{rt}", tag="mu", bufs=2)
                        nc.scalar.mul(mu_n, yn[:, 10:11], -DT_STEP / D)   # -mean(h)
                        ex2 = tail.tile([128, 1], F32, name=f"ex2_{rt}", tag="ex2", bufs=2)
                        nc.scalar.mul(ex2, p2, DT_STEP * DT_STEP / D)     # E[h^2]
                        var = tail.tile([128, 1], F32, name=f"var{rt}", tag="var", bufs=2)
                        # var = E[h^2] - mu^2 = -(mu_n*mu_n) + ex2
                        nc.vector.scalar_tensor_tensor(var, in0=mu_n, scalar=-1.0,
                                                       op0=ALU.mult, in1=mu_n,
                                                       op1=ALU.mult)
                        nc.vector.tensor_add(var, var, ex2)
                        sd = tail.tile([128, 1], F32, name=f"sd{rt}", tag="sd", bufs=2)
                        nc.scalar.activation(sd, var, AF.Sqrt, bias=eps_sb, scale=1.0)
                        inv = tail.tile([128, 1], F32, name=f"inv{rt}", tag="inv", bufs=2)
                        nc.vector.reciprocal(inv, sd)
                        qn = tail.tile([128, 1], F32, name=f"qn{rt}", tag="qn", bufs=2)
                        nc.vector.tensor_mul(qn, mu_n, inv)               # -mu*inv

                        t1 = tail.tile([128, 10], F32, name=f"t1_{rt}", tag="t1", bufs=2)
                        nc.vector.tensor_scalar_mul(t1, yn[:, 0:10], inv)
                        t2 = tail.tile([128, 10], F32, name=f"t2_{rt}", tag="t2", bufs=2)
                        nc.vector.scalar_tensor_tensor(t2, in0=w1_bc, scalar=qn,
                                                       in1=t1, op0=ALU.mult, op1=ALU.add)
                        o = tail.tile([128, 10], F32, name=f"o{rt}", tag="o", bufs=2)
                        nc.vector.tensor_add(o, t2, b2_bc)
                        nc.sync.dma_start(out=out[sl, :], in_=o)
                tp2ctx.close()

    nc.compile()
    return nc


_NC_CACHE = None


def _get_program():
    global _NC_CACHE
    if _NC_CACHE is None:
        _NC_CACHE = _build_program()
    return _NC_CACHE


def _prepare_in_maps(inputs):
    x = np.asarray(inputs["x"], dtype=np.float32)
    w_enc = np.asarray(inputs["W_enc"], dtype=np.float32)
    w_res = np.asarray(inputs["W_res"], dtype=np.float32)
    w_in = np.asarray(inputs["W_in"], dtype=np.float32)
    bias = np.asarray(inputs["bias"], dtype=np.float32)
    ln_g = np.asarray(inputs["ln_g"], dtype=np.float32)
    ln_b = np.asarray(inputs["ln_b"], dtype=np.float32)
    w_out = np.asarray(inputs["W_out"], dtype=np.float32)
    b_out = np.asarray(inputs["b_out"], dtype=np.float32)

    w_c = (w_enc.T.astype(np.float64) @ w_in.astype(np.float64)).astype(np.float32)
    w2 = w_out * ln_g[None, :]                       # [10, D]
    # state is u = h/0.2: fold 0.2 into W_res (matmul input) and readout/stats
    w_res_s = (DT_STEP * w_res.astype(np.float64)).astype(np.float32)
    w2a = np.empty((D, 11), np.float32)
    w2a[:, :10] = DT_STEP * w2.T                     # readout: gives W2 @ h.T
    w2a[:, 10] = 1.0                                 # S1 column: sum_D u
    w1v = w2.sum(axis=1).astype(np.float32)
    b2v = (w_out.astype(np.float64) @ ln_b.astype(np.float64)
           + b_out.astype(np.float64)).astype(np.float32)

    shared = {
        "w_c": np.ascontiguousarray(w_c),
        "w_res": np.ascontiguousarray(w_res_s),
        "bias": np.ascontiguousarray(bias),
        "w2a": np.ascontiguousarray(w2a),
        "w1": np.ascontiguousarray(w1v),
        "b2": np.ascontiguousarray(b2v),
    }
    in_maps = []
    for c in range(N_CORES):
        m = dict(shared)
        m["x"] = np.ascontiguousarray(x[c * R:(c + 1) * R, :])
        in_maps.append(m)
    return in_maps


def run(inputs, trace=False, tmpdir=None):
    """Run on 8 NeuronCores; returns (out [8192,10], BassKernelResults)."""
    nc = _get_program()
    in_maps = _prepare_in_maps(inputs)
    res = bass_utils.run_bass_kernel_spmd(
        nc, in_maps, core_ids=list(range(N_CORES)), trace=trace, tmpdir=tmpdir)
    outs = [np.asarray(r["out"]) for r in res.results]
    return np.concatenate(outs, axis=0), res


def kernel(**inputs):
    out, _ = run(inputs, trace=False)
    return out



# revision 7
# speedup vs baseline: 1.1587x; 1.1587x over previous
"""Trainium2 Bass kernel for nn_ChimeraNet (encoder -> 10-step Euler RNN -> LN -> readout).

Data-parallel over 8 NeuronCores: each core gets 1024 rows of the batch and a
replicated set of (host-prefolded) weights.

Math (per core, R=1024 rows, D=1024), with u = h/0.2 so the update is
    u_{t+1} = 0.8*u_t + tanh(u_t @ (0.2 W_res) + drive_in),   u_1 = tanh(drive_in)

fp8 fast path: the recurrent matmul runs in fp8-e4m3 DoubleRow mode (K=256 per
instruction at 0.5 cycles/row -> ~4x the fp32r rate).  Everything is kept in a
scaled PSUM domain: psum = S*(u @ 0.2*W_res + drive_in) with S=32, where
  - W8    = e4m3(S * 0.2 * W_res), stationary tiles [128, 8, D]
  - u8    = e4m3(u) cast each step from the fp16 state (DVE/gpsimd copies)
  - drive_in enters PSUM via a DoubleRow identity matmul on a hi/lo fp8 pair
    (dHi = e4m3(S*drive_in), dLo = e4m3(S*drive_in - dHi); error ~ (2%)^2)
The tanh then reads psum pairs [128,1024] with scale=1/S on the ACT engine and
writes v in fp16; the DVE keeps the real state u in fp16 (u' = 0.8u + v).
LayerNorm+readout are folded as in the f32r baseline but run in fp16 (exact
ones-column for S1, squares via DVE, stats chain in f32).

Encoder x @ (W_enc.T W_in) runs in bf16 (same PE rate as f32r, half the DMA).
"""

import os
import sys

import numpy as np
import ml_dtypes

try:
    import concourse.bass as bass  # noqa: F401
except ImportError:  # pragma: no cover - fresh grading env without PYTHONPATH
    for p in ("/root/.axon_site", "/root/.axon_site/_ro/trn_rl_repo",
              "/root/.axon_site/_ro/pypackages", "/opt/trn_rl_repo"):
        if os.path.isdir(p) and p not in sys.path:
            sys.path.append(p)
    import concourse.bass as bass

from contextlib import ExitStack

import concourse.tile as tile
from concourse import bacc, bass_utils, mybir
from concourse.masks import make_identity

N_CORES = 8
B = 8192
R = B // N_CORES        # rows per core
D = 1024                # latent dim
KX = 784                # encoder input dim
DT_STEP = 0.2
STEPS = 10
EPS = 1e-5
S = 32.0                # fp8 psum domain scale

F32 = mybir.dt.float32
BF16 = mybir.dt.bfloat16
FP16 = mybir.dt.float16
E4 = mybir.dt.float8e4
AF = mybir.ActivationFunctionType
ALU = mybir.AluOpType
DR = mybir.MatmulPerfMode.DoubleRow

KD = D // 128           # 8 k/m tiles over D
NS = R // 512           # 2 moving-dim slices of 512
KXT = [128] * 6 + [16]  # 784 = 6*128 + 16
NWARM = 6               # PE warmup matmuls (HAM un-throttle during DMA wait)

E4NP = ml_dtypes.float8_e4m3
BF16NP = ml_dtypes.bfloat16


def _build_program():
    nc = bacc.Bacc("TRN2", target_bir_lowering=False, debug=False)

    x16 = nc.dram_tensor("x16", [R, KX], BF16, kind="ExternalInput").ap()
    wc16 = nc.dram_tensor("wc16", [KX, D], BF16, kind="ExternalInput").ap()
    w8 = nc.dram_tensor("w8", [128, KD, D], E4, kind="ExternalInput").ap()
    bias = nc.dram_tensor("bias", [D], F32, kind="ExternalInput").ap()
    w2a = nc.dram_tensor("w2a", [D, 11], FP16, kind="ExternalInput").ap()
    w1 = nc.dram_tensor("w1", [10], F32, kind="ExternalInput").ap()
    b2 = nc.dram_tensor("b2", [10], F32, kind="ExternalInput").ap()
    out = nc.dram_tensor("out", [R, 10], F32, kind="ExternalOutput").ap()

    with tile.TileContext(nc) as tc, ExitStack() as ctx:
        state = ctx.enter_context(tc.tile_pool(name="state", bufs=1))
        consts = ctx.enter_context(tc.tile_pool(name="consts", bufs=1))
        wres_pool = ctx.enter_context(tc.tile_pool(name="wres", bufs=1))

        # persistent SBUF state (all transposed: D on partitions, rows free)
        u_sb = [state.tile([128, KD, R], FP16, name=f"u{b}", tag=f"u{b}")
                for b in range(2)]
        u8_sb = [state.tile([128, KD, R], E4, name=f"u8{b}", tag=f"u8{b}")
                 for b in range(2)]
        v_sb = state.tile([128, KD, R], FP16, name="v", tag="v")
        sq_sb = state.tile([128, KD, R], FP16, name="sq", tag="sq")
        dinHL = state.tile([128, 2 * KD, R], E4, name="dinHL", tag="dinHL")
        w8_sb = wres_pool.tile([128, KD, D], E4, name="w8sb", tag="w8sb")

        with ExitStack() as mmctx:
            # PE warmup: dependency-free fp32 matmuls starting at t~0 pull
            # the HAM clock gate to 8/8 while the input DMAs are in flight.
            warmctx = ExitStack()
            warm_psum = warmctx.enter_context(
                tc.tile_pool(name="warm", bufs=1, space="PSUM"))
            warm_src = consts.tile([128, 256], F32)
            nc.vector.memset(warm_src, 0.01)
            warm_sb = consts.tile([128, 1], F32)
            for w in range(NWARM):
                wp = warm_psum.tile([128, 512], F32, name=f"warm{w}", tag="wm")
                nc.tensor.matmul(wp[:, :256], lhsT=warm_src[:, :128], rhs=warm_src,
                                 start=True, stop=True)
                if w == NWARM - 1:
                    nc.vector.tensor_copy(warm_sb, wp[:, :1])  # keep-alive

            ident = consts.tile([128, 128], F32)
            make_identity(nc, ident)
            identb = consts.tile([128, 128], BF16)
            nc.vector.tensor_copy(identb, ident)
            idpair = consts.tile([128, 2, 128], E4)
            nc.vector.tensor_copy(idpair[:, 0, :], ident)
            nc.vector.tensor_copy(idpair[:, 1, :], ident)
            bias_sb = consts.tile([128, KD], F32)
            nc.gpsimd.dma_start(out=bias_sb, in_=bias.rearrange("(m p) -> p m", p=128))

            # W8 arrives on the gpsimd queue while x streams on sync.
            nc.gpsimd.dma_start(out=w8_sb, in_=w8)

            # ------------ encoder: x -> x.T (bf16), drive = x @ W_c + bias ----
            drive_pool = ExitStack()
            dp = drive_pool.enter_context(tc.tile_pool(name="drv", bufs=1))
            drive = dp.tile([128, KD, R], F32, name="drive", tag="drive")

            with ExitStack() as enc:
                xn_pool = enc.enter_context(tc.tile_pool(name="xn", bufs=4))
                xt_pool = enc.enter_context(tc.tile_pool(name="xt", bufs=1))
                wc_pool = enc.enter_context(tc.tile_pool(name="wc", bufs=1))
                etp = enc.enter_context(
                    tc.tile_pool(name="etp", bufs=3, space="PSUM"))
                eps_pool = enc.enter_context(
                    tc.tile_pool(name="emm", bufs=4, space="PSUM"))

                xt_big = xt_pool.tile([128, len(KXT), R], BF16, name="xt_big")
                wc_sb = [wc_pool.tile([128, D], BF16, name=f"wc{k}", tag=f"wc{k}")
                         for k in range(len(KXT))]
                for k, kw in enumerate(KXT):
                    nc.scalar.dma_start(out=wc_sb[k][:kw, :],
                                        in_=wc16[k * 128:k * 128 + kw, :])

                def transpose_rt(rt):
                    xn = xn_pool.tile([128, KX], BF16, name=f"xn{rt}", tag="xn")
                    nc.sync.dma_start(out=xn, in_=x16[rt * 128:(rt + 1) * 128, :])
                    rsl = slice(rt * 128, (rt + 1) * 128)
                    # dependency-free matmul BEFORE the transposes: runs during
                    # this tile's DMA wait, keeps the HAM window at 8/8
                    wp0 = warm_psum.tile([128, 512], F32, name=f"wmh{rt}", tag="wm")
                    nc.tensor.matmul(wp0[:, :256], lhsT=warm_src[:, :128],
                                     rhs=warm_src, start=True, stop=True)
                    for kp in range(3):
                        pt = etp.tile([128, 256], BF16, name=f"pt{rt}_{kp}", tag="tp")
                        for h in range(2):
                            k = 2 * kp + h
                            nc.tensor.transpose(pt[:, h * 128:(h + 1) * 128],
                                                xn[:, k * 128:(k + 1) * 128], identb)
                        src = pt.rearrange("p (two c) -> p two c", two=2)
                        dst = xt_big[:, 2 * kp:2 * kp + 2, rsl]
                        if kp % 2 == 0:
                            nc.scalar.copy(dst, src)
                        else:
                            nc.vector.tensor_copy(dst, src)
                    pt = etp.tile([128, 256], BF16, name=f"pt{rt}_3", tag="tp")
                    nc.tensor.transpose(pt[:16, :128], xn[:, 768:784], identb)
                    nc.vector.tensor_copy(xt_big[:16, 6, rsl], pt[:16, :128])

                def encoder_mms(n):
                    sl = slice(n * 512, (n + 1) * 512)
                    for m in range(KD):
                        ps = eps_pool.tile([128, 512], F32, name=f"eps{n}_{m}",
                                           tag="emm")
                        for k, kw in enumerate(KXT):
                            nc.tensor.matmul(
                                ps,
                                lhsT=wc_sb[k][:kw, m * 128:(m + 1) * 128],
                                rhs=xt_big[:kw, k, sl],
                                start=(k == 0), stop=(k == len(KXT) - 1))
                        dsl = drive[:, m, sl]
                        nc.scalar.activation(dsl, ps, AF.Identity,
                                             bias=bias_sb[:, m:m + 1], scale=1.0)
                        # u_1 = v_0 = tanh(drive_in)  (fp16 state)
                        nc.scalar.activation(u_sb[1][:, m, sl], dsl, AF.Tanh)
                        # dHi = e4m3(S*drive) = 33*drive - drive; dLo = S*drive - dHi
                        nc.vector.scalar_tensor_tensor(
                            dinHL[:, 2 * m, sl], in0=dsl, scalar=S + 1.0,
                            op0=ALU.mult, in1=dsl, op1=ALU.subtract)
                        nc.vector.scalar_tensor_tensor(
                            dinHL[:, 2 * m + 1, sl], in0=dsl, scalar=S,
                            op0=ALU.mult, in1=dinHL[:, 2 * m, sl], op1=ALU.subtract)
                        nc.gpsimd.tensor_copy(u8_sb[1][:, m, sl], u_sb[1][:, m, sl])

                # interleave: the n=0 encoder matmuls run while rows 4-7 DMA in
                for rt in range(4):
                    transpose_rt(rt)
                encoder_mms(0)
                for rt in range(4, 8):
                    transpose_rt(rt)
                encoder_mms(1)
            drive_pool.close()
            warmctx.close()

            # ------------ Euler integration loop (fp8 DoubleRow) --------------
            loopctx = ExitStack()
            psum = loopctx.enter_context(
                tc.tile_pool(name="mm", bufs=3, space="PSUM"))

            for s in range(1, STEPS):
                cur, nxt = s % 2, (s + 1) % 2
                for n in range(NS):
                    sl = slice(n * 512, (n + 1) * 512)
                    for mp in range(KD // 2):
                        ps = psum.tile([128, 1024], F32, name=f"ps{s}_{n}_{mp}",
                                       tag="mm")
                        for half in range(2):
                            m = 2 * mp + half
                            psh = ps[:, half * 512:(half + 1) * 512]
                            nc.tensor.matmul(psh, lhsT=idpair,
                                             rhs=dinHL[:, 2 * m:2 * m + 2, sl],
                                             start=True, stop=False, perf_mode=DR)
                            for k2 in range(KD // 2):
                                nc.tensor.matmul(
                                    psh,
                                    lhsT=w8_sb[:, 2 * k2:2 * k2 + 2,
                                               m * 128:(m + 1) * 128],
                                    rhs=u8_sb[cur][:, 2 * k2:2 * k2 + 2, sl],
                                    start=False, stop=(k2 == KD // 2 - 1),
                                    perf_mode=DR)
                        # v = tanh(psum/S) for both halves in one ACT op
                        nc.scalar.activation(
                            v_sb[:, 2 * mp:2 * mp + 2, sl],
                            ps.rearrange("p (two c) -> p two c", two=2),
                            AF.Tanh, scale=1.0 / S)
                        for half in range(2):
                            m = 2 * mp + half
                            nc.vector.scalar_tensor_tensor(
                                u_sb[nxt][:, m, sl], in0=u_sb[cur][:, m, sl],
                                scalar=1.0 - DT_STEP, op0=ALU.mult,
                                in1=v_sb[:, m, sl], op1=ALU.add)
                            cast_eng = nc.vector if half == 0 else nc.gpsimd
                            cast_eng.tensor_copy(u8_sb[nxt][:, m, sl],
                                                 u_sb[nxt][:, m, sl])
                            if s == STEPS - 1:
                                nc.vector.tensor_mul(sq_sb[:, m, sl],
                                                     u_sb[nxt][:, m, sl],
                                                     u_sb[nxt][:, m, sl])

            loopctx.close()
            uf = u_sb[STEPS % 2]

            # ------------ tail: LN stats + readout (fp16 matmuls) -------------
            tail = ctx.enter_context(tc.tile_pool(name="tail", bufs=1))
            tmm = mmctx.enter_context(
                tc.tile_pool(name="tmm", bufs=2, space="PSUM"))

            ones_f32 = tail.tile([128, 1], F32)
            nc.vector.memset(ones_f32, 1.0)
            ones_sb = tail.tile([128, 1], FP16)
            nc.scalar.copy(ones_sb, ones_f32)
            eps_sb = tail.tile([128, 1], F32)
            nc.vector.memset(eps_sb, EPS)
            # w2a = [0.2*W2.T | ones] : readout weights + S1 column
            w2a_sb = tail.tile([128, KD, 11], FP16)
            nc.gpsimd.dma_start(out=w2a_sb,
                                in_=w2a.rearrange("(k p) o -> p k o", p=128))
            w1_bc = tail.tile([128, 10], F32)
            nc.gpsimd.dma_start(out=w1_bc, in_=bass.AP(tensor=w1.tensor, offset=w1.offset,
                                                       ap=[[0, 128]] + list(w1.ap)))
            b2_bc = tail.tile([128, 10], F32)
            nc.gpsimd.dma_start(out=b2_bc, in_=bass.AP(tensor=b2.tensor, offset=b2.offset,
                                                       ap=[[0, 128]] + list(b2.ap)))

            s2_sb = tail.tile([1, R], F32)
            y_sb = tail.tile([11, R], F32)

            # per-n readout matmuls, with the per-row-tile stat/combine chains
            # interleaved so the n=0 half finishes while n=1 runs.
            # y_sb rows 0-9 = 0.2*(W2 @ u.T) = W2 @ h.T;  row 10 = sum_D u.
            tp2ctx = ExitStack()
            tp2 = tp2ctx.enter_context(
                tc.tile_pool(name="tp2", bufs=4, space="PSUM"))
            for n in range(NS):
                sl = slice(n * 512, (n + 1) * 512)
                yp = tmm.tile([128, 512], F32, name=f"yp{n}", tag="tmm")
                for k in range(KD):
                    nc.tensor.matmul(yp[:11, :], lhsT=w2a_sb[:, k, :],
                                     rhs=uf[:, k, sl],
                                     start=(k == 0), stop=(k == KD - 1))
                nc.scalar.copy(y_sb[:, sl], yp[:11, :])
                s2 = tmm.tile([128, 512], F32, name=f"s2p{n}", tag="tmm")
                for k in range(KD):
                    nc.tensor.matmul(s2[:1, :], lhsT=ones_sb,
                                     rhs=sq_sb[:, k, sl],
                                     start=(k == 0), stop=(k == KD - 1))
                nc.scalar.copy(s2_sb[:, sl], s2[:1, :])

                for rt in range(n * 4, (n + 1) * 4):
                    sl = slice(rt * 128, (rt + 1) * 128)
                    yn = tp2.tile([128, 11], F32, name=f"yn{rt}", tag="st")
                    nc.tensor.transpose(yn, y_sb[:, sl], ident[:11, :11])
                    p2 = tp2.tile([128, 1], F32, name=f"p2_{rt}", tag="st")
                    nc.tensor.transpose(p2, s2_sb[:, sl], ident[:1, :1])
                    mu_n = tail.tile([128, 1], F32, name=f"mu{rt}", tag="mu", bufs=2)
                    nc.scalar.mul(mu_n, yn[:, 10:11], -DT_STEP / D)   # -mean(h)
                    ex2 = tail.tile([128, 1], F32, name=f"ex2_{rt}", tag="ex2", bufs=2)
                    nc.scalar.mul(ex2, p2, DT_STEP * DT_STEP / D)     # E[h^2]
                    var = tail.tile([128, 1], F32, name=f"var{rt}", tag="var", bufs=2)
                    # var = E[h^2] - mu^2 = -(mu_n*mu_n) + ex2
                    nc.vector.scalar_tensor_tensor(var, in0=mu_n, scalar=-1.0,
                                                   op0=ALU.mult, in1=mu_n,
                                                   op1=ALU.mult)
                    nc.vector.tensor_add(var, var, ex2)
                    sd = tail.tile([128, 1], F32, name=f"sd{rt}", tag="sd", bufs=2)
                    nc.scalar.activation(sd, var, AF.Sqrt, bias=eps_sb, scale=1.0)
                    inv = tail.tile([128, 1], F32, name=f"inv{rt}", tag="inv", bufs=2)
                    nc.vector.reciprocal(inv, sd)
                    qn = tail.tile([128, 1], F32, name=f"qn{rt}", tag="qn", bufs=2)
                    nc.vector.tensor_mul(qn, mu_n, inv)               # -mu*inv

                    t1 = tail.tile([128, 10], F32, name=f"t1_{rt}", tag="t1", bufs=2)
                    nc.vector.tensor_scalar_mul(t1, yn[:, 0:10], inv)
                    t2 = tail.tile([128, 10], F32, name=f"t2_{rt}", tag="t2", bufs=2)
                    nc.vector.scalar_tensor_tensor(t2, in0=w1_bc, scalar=qn,
                                                   in1=t1, op0=ALU.mult, op1=ALU.add)
                    o = tail.tile([128, 10], F32, name=f"o{rt}", tag="o", bufs=2)
                    nc.vector.tensor_add(o, t2, b2_bc)
                    nc.sync.dma_start(out=out[sl, :], in_=o)
            tp2ctx.close()

    nc.compile()
    return nc


_NC_CACHE = None


def _get_program():
    global _NC_CACHE
    if _NC_CACHE is None:
        _NC_CACHE = _build_program()
    return _NC_CACHE


def _prepare_in_maps(inputs):
    x = np.asarray(inputs["x"], dtype=np.float32)
    w_enc = np.asarray(inputs["W_enc"], dtype=np.float32)
    w_res = np.asarray(inputs["W_res"], dtype=np.float32)
    w_in = np.asarray(inputs["W_in"], dtype=np.float32)
    bias = np.asarray(inputs["bias"], dtype=np.float32)
    ln_g = np.asarray(inputs["ln_g"], dtype=np.float32)
    ln_b = np.asarray(inputs["ln_b"], dtype=np.float32)
    w_out = np.asarray(inputs["W_out"], dtype=np.float32)
    b_out = np.asarray(inputs["b_out"], dtype=np.float32)

    w_c = (w_enc.T.astype(np.float64) @ w_in.astype(np.float64)).astype(np.float32)
    w2 = w_out * ln_g[None, :]                       # [10, D]
    # fp8 stationary weights in the S-scaled domain: [128, KD, D],
    # element (p, ks, m) = S*0.2*W_res[ks*128+p, m]
    w8 = (S * DT_STEP * w_res).astype(E4NP).reshape(KD, 128, D).transpose(1, 0, 2)
    w2a = np.empty((D, 11), np.float32)
    w2a[:, :10] = DT_STEP * w2.T                     # readout: gives W2 @ h.T
    w2a[:, 10] = 1.0                                 # S1 column: sum_D u
    w1v = w2.sum(axis=1).astype(np.float32)
    b2v = (w_out.astype(np.float64) @ ln_b.astype(np.float64)
           + b_out.astype(np.float64)).astype(np.float32)

    shared = {
        "wc16": np.ascontiguousarray(w_c.astype(BF16NP)),
        "w8": np.ascontiguousarray(w8),
        "bias": np.ascontiguousarray(bias),
        "w2a": np.ascontiguousarray(w2a.astype(np.float16)),
        "w1": np.ascontiguousarray(w1v),
        "b2": np.ascontiguousarray(b2v),
    }
    in_maps = []
    for c in range(N_CORES):
        m = dict(shared)
        m["x16"] = np.ascontiguousarray(x[c * R:(c + 1) * R, :].astype(BF16NP))
        in_maps.append(m)
    return in_maps


def run(inputs, trace=False, tmpdir=None):
    """Run on 8 NeuronCores; returns (out [8192,10], BassKernelResults)."""
    nc = _get_program()
    in_maps = _prepare_in_maps(inputs)
    res = bass_utils.run_bass_kernel_spmd(
        nc, in_maps, core_ids=list(range(N_CORES)), trace=trace, tmpdir=tmpdir)
    outs = [np.asarray(r["out"]) for r in res.results]
    return np.concatenate(outs, axis=0), res


def kernel(**inputs):
    out, _ = run(inputs, trace=False)
    return out


# revision 19
# speedup vs baseline: 1.3335x; 1.1508x over previous
"""Trainium2 Bass kernel for nn_ChimeraNet (encoder -> 10-step Euler RNN -> LN -> readout).

Data-parallel over 8 NeuronCores: each core gets 1024 rows of the batch and a
replicated set of (host-prefolded) weights.

Math (per core, R=1024 rows, D=1024), with u = h/0.2 so the update is
    u_{t+1} = 0.8*u_t + tanh(u_t @ (0.2 W_res) + drive_in),   u_1 = tanh(drive_in)

fp8 fast path: the recurrent matmul runs in fp8-e4m3 DoubleRow mode (K=256 per
instruction at 0.5 cycles/row -> ~4x the fp32r rate).  Everything is kept in a
scaled PSUM domain: psum = S*(u @ 0.2*W_res + drive_in) with S=32, where
  - W8    = e4m3(S * 0.2 * W_res), stationary tiles [128, 8, D]
  - u8    = e4m3(u) cast each step from the fp16 state (DVE/gpsimd copies)
  - drive_in enters PSUM via a DoubleRow identity matmul on a hi/lo fp8 pair
    (dHi = e4m3(S*drive_in), dLo = e4m3(S*drive_in - dHi); error ~ (2%)^2)
The tanh then reads psum pairs [128,1024] with scale=1/S on the ACT engine and
writes v in fp16; the DVE keeps the real state u in fp16 (u' = 0.8u + v).
LayerNorm+readout are folded as in the f32r baseline but run in fp16 (exact
ones-column for S1, squares via DVE, stats chain in f32).

Encoder x @ (W_enc.T W_in) runs in bf16 (same PE rate as f32r, half the DMA).
"""

import os
import sys

import numpy as np
import ml_dtypes

try:
    import concourse.bass as bass  # noqa: F401
except ImportError:  # pragma: no cover - fresh grading env without PYTHONPATH
    for p in ("/root/.axon_site", "/root/.axon_site/_ro/trn_rl_repo",
              "/root/.axon_site/_ro/pypackages", "/opt/trn_rl_repo"):
        if os.path.isdir(p) and p not in sys.path:
            sys.path.append(p)
    import concourse.bass as bass

from contextlib import ExitStack

import concourse.tile as tile
from concourse import bacc, bass_utils, mybir
from concourse.masks import make_identity

N_CORES = 8
B = 8192
R = B // N_CORES        # rows per core
D = 1024                # latent dim
KX = 784                # encoder input dim
DT_STEP = 0.2
STEPS = 10
EPS = 1e-5
S = 32.0                # fp8 psum domain scale

F32 = mybir.dt.float32
BF16 = mybir.dt.bfloat16
FP16 = mybir.dt.float16
E4 = mybir.dt.float8e4
AF = mybir.ActivationFunctionType
ALU = mybir.AluOpType
DR = mybir.MatmulPerfMode.DoubleRow

KD = D // 128           # 8 k/m tiles over D
NS = R // 512           # 2 moving-dim slices of 512
KXT = [128] * 6 + [16]  # 784 = 6*128 + 16
NWARM = 6               # PE warmup matmuls (HAM un-throttle during DMA wait)

E4NP = ml_dtypes.float8_e4m3
BF16NP = ml_dtypes.bfloat16


def _build_program():
    nc = bacc.Bacc("TRN2", target_bir_lowering=False, debug=False)

    x16 = nc.dram_tensor("x16", [R, KX], BF16, kind="ExternalInput").ap()
    wc16 = nc.dram_tensor("wc16", [KX, D], BF16, kind="ExternalInput").ap()
    w8 = nc.dram_tensor("w8", [128, KD, D], E4, kind="ExternalInput").ap()
    bias = nc.dram_tensor("bias", [D], F32, kind="ExternalInput").ap()
    w2a = nc.dram_tensor("w2a", [D, 11], F32, kind="ExternalInput").ap()
    w1 = nc.dram_tensor("w1", [10], F32, kind="ExternalInput").ap()
    b2 = nc.dram_tensor("b2", [10], F32, kind="ExternalInput").ap()
    out = nc.dram_tensor("out", [R, 10], F32, kind="ExternalOutput").ap()

    with tile.TileContext(nc) as tc, ExitStack() as ctx:
        state = ctx.enter_context(tc.tile_pool(name="state", bufs=1))
        consts = ctx.enter_context(tc.tile_pool(name="consts", bufs=1))
        wres_pool = ctx.enter_context(tc.tile_pool(name="wres", bufs=1))

        # persistent SBUF state (all transposed: D on partitions, rows free).
        # u/v in f32: DVE ALU ops run at full rate in f32 (fp16 is half rate).
        # u tiles are f32r so the tail readout matmul can consume them directly.
        F32R = mybir.dt.float32r
        u_sb = [state.tile([128, KD, R], F32R, name=f"u{b}", tag=f"u{b}")
                for b in range(2)]
        u8_sb = [state.tile([128, KD, R], E4, name=f"u8{b}", tag=f"u8{b}")
                 for b in range(2)]
        v_sb = state.tile([128, KD, R], F32, name="v", tag="v")
        sq_sb = state.tile([128, KD, R], FP16, name="sq", tag="sq")
        dinHL = state.tile([128, 2 * KD, R], E4, name="dinHL", tag="dinHL")
        w8_sb = wres_pool.tile([128, KD, D], E4, name="w8sb", tag="w8sb")

        with ExitStack() as mmctx:
            # PE warmup: dependency-free fp32 matmuls starting at t~0 pull
            # the HAM clock gate to 8/8 while the input DMAs are in flight.
            warmctx = ExitStack()
            warm_psum = warmctx.enter_context(
                tc.tile_pool(name="warm", bufs=1, space="PSUM"))
            warm_src = consts.tile([128, 256], F32)
            nc.vector.memset(warm_src, 0.01)
            warm_sb = consts.tile([128, 1], F32)
            for w in range(NWARM):
                wp = warm_psum.tile([128, 512], F32, name=f"warm{w}", tag="wm")
                nc.tensor.matmul(wp[:, :256], lhsT=warm_src[:, :128], rhs=warm_src,
                                 start=True, stop=True)
                if w == NWARM - 1:
                    nc.vector.tensor_copy(warm_sb, wp[:, :1])  # keep-alive

            ident = consts.tile([128, 128], F32)
            make_identity(nc, ident)
            identb = consts.tile([128, 128], BF16)
            nc.vector.tensor_copy(identb, ident)
            idpair = consts.tile([128, 2, 128], E4)
            nc.vector.tensor_copy(idpair[:, 0, :], ident)
            nc.vector.tensor_copy(idpair[:, 1, :], ident)
            bias_sb = consts.tile([128, KD], F32)
            nc.gpsimd.dma_start(out=bias_sb, in_=bias.rearrange("(m p) -> p m", p=128))

            # W8 arrives on the gpsimd queue while x streams on sync.
            nc.gpsimd.dma_start(out=w8_sb, in_=w8)

            # ------------ encoder: x -> x.T (bf16), drive = x @ W_c + bias ----
            # drive tiles rotate through a small pool: each (m,n) tile is dead
            # as soon as its tanh + dHi/dLo consumers have run.
            drive_pool = ExitStack()
            dp = drive_pool.enter_context(tc.tile_pool(name="drv", bufs=6))

            with ExitStack() as enc:
                xn_pool = enc.enter_context(tc.tile_pool(name="xn", bufs=4))
                xt_pool = enc.enter_context(tc.tile_pool(name="xt", bufs=1))
                wc_pool = enc.enter_context(tc.tile_pool(name="wc", bufs=1))
                etp = enc.enter_context(
                    tc.tile_pool(name="etp", bufs=3, space="PSUM"))
                eps_pool = enc.enter_context(
                    tc.tile_pool(name="emm", bufs=4, space="PSUM"))

                xt_big = xt_pool.tile([128, len(KXT), R], BF16, name="xt_big")
                wc_sb = [wc_pool.tile([128, D], BF16, name=f"wc{k}", tag=f"wc{k}")
                         for k in range(len(KXT))]
                for k, kw in enumerate(KXT):
                    nc.scalar.dma_start(out=wc_sb[k][:kw, :],
                                        in_=wc16[k * 128:k * 128 + kw, :])

                def transpose_rt(rt):
                    xn = xn_pool.tile([128, KX], BF16, name=f"xn{rt}", tag="xn")
                    nc.sync.dma_start(out=xn, in_=x16[rt * 128:(rt + 1) * 128, :])
                    rsl = slice(rt * 128, (rt + 1) * 128)
                    # dependency-free matmul BEFORE the transposes: runs during
                    # this tile's DMA wait, keeps the HAM window at 8/8
                    wp0 = warm_psum.tile([128, 512], F32, name=f"wmh{rt}", tag="wm")
                    nc.tensor.matmul(wp0[:, :256], lhsT=warm_src[:, :128],
                                     rhs=warm_src, start=True, stop=True)
                    for kp in range(3):
                        pt = etp.tile([128, 256], BF16, name=f"pt{rt}_{kp}", tag="tp")
                        for h in range(2):
                            k = 2 * kp + h
                            nc.tensor.transpose(pt[:, h * 128:(h + 1) * 128],
                                                xn[:, k * 128:(k + 1) * 128], identb)
                        src = pt.rearrange("p (two c) -> p two c", two=2)
                        dst = xt_big[:, 2 * kp:2 * kp + 2, rsl]
                        if kp % 2 == 0:
                            nc.scalar.copy(dst, src)
                        else:
                            nc.vector.tensor_copy(dst, src)
                    pt = etp.tile([128, 256], BF16, name=f"pt{rt}_3", tag="tp")
                    nc.tensor.transpose(pt[:16, :128], xn[:, 768:784], identb)
                    nc.vector.tensor_copy(xt_big[:16, 6, rsl], pt[:16, :128])

                def encoder_mms(n):
                    sl = slice(n * 512, (n + 1) * 512)
                    for m in range(KD):
                        ps = eps_pool.tile([128, 512], F32, name=f"eps{n}_{m}",
                                           tag="emm")
                        for k, kw in enumerate(KXT):
                            nc.tensor.matmul(
                                ps,
                                lhsT=wc_sb[k][:kw, m * 128:(m + 1) * 128],
                                rhs=xt_big[:kw, k, sl],
                                start=(k == 0), stop=(k == len(KXT) - 1))
                        dsl = dp.tile([128, 512], F32, name=f"dr{n}_{m}", tag="dr")
                        nc.scalar.activation(dsl, ps, AF.Identity,
                                             bias=bias_sb[:, m:m + 1], scale=1.0)
                        # u_1 = v_0 = tanh(drive_in)  (f32 state)
                        nc.scalar.activation(u_sb[1][:, m, sl], dsl, AF.Tanh)
                        # dHi = e4m3(S*drive) = 33*drive - drive; dLo = S*drive - dHi
                        nc.vector.scalar_tensor_tensor(
                            dinHL[:, 2 * m, sl], in0=dsl, scalar=S + 1.0,
                            op0=ALU.mult, in1=dsl, op1=ALU.subtract)
                        nc.vector.scalar_tensor_tensor(
                            dinHL[:, 2 * m + 1, sl], in0=dsl, scalar=S,
                            op0=ALU.mult, in1=dinHL[:, 2 * m, sl], op1=ALU.subtract)
                        if m % 2 == 0:
                            nc.scalar.copy(u8_sb[1][:, m, sl], u_sb[1][:, m, sl])
                        else:
                            nc.gpsimd.tensor_copy(u8_sb[1][:, m, sl],
                                                  u_sb[1][:, m, sl])

                # interleave: the n=0 encoder matmuls run while rows 4-7 DMA in
                for rt in range(4):
                    transpose_rt(rt)
                encoder_mms(0)
                for rt in range(4, 8):
                    transpose_rt(rt)
                encoder_mms(1)
            drive_pool.close()
            warmctx.close()

            # ------------ Euler integration loop (fp8 DoubleRow) --------------
            loopctx = ExitStack()
            psum = loopctx.enter_context(
                tc.tile_pool(name="mm", bufs=3, space="PSUM"))

            for s in range(1, STEPS):
                cur, nxt = s % 2, (s + 1) % 2
                for n in range(NS):
                    sl = slice(n * 512, (n + 1) * 512)
                    for mp in range(KD // 2):
                        ps = psum.tile([128, 1024], F32, name=f"ps{s}_{n}_{mp}",
                                       tag="mm")
                        for half in range(2):
                            m = 2 * mp + half
                            psh = ps[:, half * 512:(half + 1) * 512]
                            nc.tensor.matmul(psh, lhsT=idpair,
                                             rhs=dinHL[:, 2 * m:2 * m + 2, sl],
                                             start=True, stop=False, perf_mode=DR)
                            for k2 in range(KD // 2):
                                nc.tensor.matmul(
                                    psh,
                                    lhsT=w8_sb[:, 2 * k2:2 * k2 + 2,
                                               m * 128:(m + 1) * 128],
                                    rhs=u8_sb[cur][:, 2 * k2:2 * k2 + 2, sl],
                                    start=False, stop=(k2 == KD // 2 - 1),
                                    perf_mode=DR)
                        # v = tanh(psum/S) for both halves in one ACT op
                        nc.scalar.activation(
                            v_sb[:, 2 * mp:2 * mp + 2, sl],
                            ps.rearrange("p (two c) -> p two c", two=2),
                            AF.Tanh, scale=1.0 / S)
                        for half in range(2):
                            m = 2 * mp + half
                            nc.vector.scalar_tensor_tensor(
                                u_sb[nxt][:, m, sl], in0=u_sb[cur][:, m, sl],
                                scalar=1.0 - DT_STEP, op0=ALU.mult,
                                in1=v_sb[:, m, sl], op1=ALU.add)
                            if s < STEPS - 1:
                                # cast for the next step's matmuls (DVE does only
                                # the stt; casts go to ACT / gpsimd)
                                if half == 0:
                                    nc.scalar.copy(u8_sb[nxt][:, m, sl],
                                                   u_sb[nxt][:, m, sl])
                                else:
                                    nc.gpsimd.tensor_copy(u8_sb[nxt][:, m, sl],
                                                          u_sb[nxt][:, m, sl])
                            else:
                                # last step: u8 is dead; compute LN squares instead
                                if half == 0:
                                    nc.scalar.square(sq_sb[:, m, sl],
                                                     u_sb[nxt][:, m, sl])
                                else:
                                    nc.vector.tensor_mul(sq_sb[:, m, sl],
                                                         u_sb[nxt][:, m, sl],
                                                         u_sb[nxt][:, m, sl])

            loopctx.close()
            uf = u_sb[STEPS % 2]

            # ------------ tail: LN stats + readout (fp16 matmuls) -------------
            tail = ctx.enter_context(tc.tile_pool(name="tail", bufs=1))
            tmm = mmctx.enter_context(
                tc.tile_pool(name="tmm", bufs=2, space="PSUM"))

            ones_f32 = tail.tile([128, 1], F32)
            nc.vector.memset(ones_f32, 1.0)
            ones_sb = tail.tile([128, 1], FP16)
            nc.scalar.copy(ones_sb, ones_f32)
            eps_sb = tail.tile([128, 1], F32)
            nc.vector.memset(eps_sb, EPS)
            # w2a = [0.2*W2.T | ones] : readout weights + S1 column
            w2a_sb = tail.tile([128, KD, 11], F32R)
            nc.gpsimd.dma_start(out=w2a_sb,
                                in_=w2a.rearrange("(k p) o -> p k o",
                                                  p=128).bitcast(F32R))
            w1_bc = tail.tile([128, 10], F32)
            nc.gpsimd.dma_start(out=w1_bc, in_=bass.AP(tensor=w1.tensor, offset=w1.offset,
                                                       ap=[[0, 128]] + list(w1.ap)))
            b2_bc = tail.tile([128, 10], F32)
            nc.gpsimd.dma_start(out=b2_bc, in_=bass.AP(tensor=b2.tensor, offset=b2.offset,
                                                       ap=[[0, 128]] + list(b2.ap)))

            s2_sb = tail.tile([1, R], F32)
            y_sb = tail.tile([11, R], F32)

            # per-n readout matmuls, with the per-row-tile stat/combine chains
            # interleaved so the n=0 half finishes while n=1 runs.
            # y_sb rows 0-9 = 0.2*(W2 @ u.T) = W2 @ h.T;  row 10 = sum_D u.
            tp2ctx = ExitStack()
            tp2 = tp2ctx.enter_context(
                tc.tile_pool(name="tp2", bufs=4, space="PSUM"))
            for n in range(NS):
                sl = slice(n * 512, (n + 1) * 512)
                yp = tmm.tile([128, 512], F32, name=f"yp{n}", tag="tmm")
                for k in range(KD):
                    nc.tensor.matmul(yp[:11, :], lhsT=w2a_sb[:, k, :],
                                     rhs=uf[:, k, sl],
                                     start=(k == 0), stop=(k == KD - 1))
                nc.scalar.copy(y_sb[:, sl], yp[:11, :])
                s2 = tmm.tile([128, 512], F32, name=f"s2p{n}", tag="tmm")
                for k in range(KD):
                    nc.tensor.matmul(s2[:1, :], lhsT=ones_sb,
                                     rhs=sq_sb[:, k, sl],
                                     start=(k == 0), stop=(k == KD - 1))
                nc.scalar.copy(s2_sb[:, sl], s2[:1, :])

                for rt in range(n * 4, (n + 1) * 4):
                    sl = slice(rt * 128, (rt + 1) * 128)
                    yn = tp2.tile([128, 11], F32, name=f"yn{rt}", tag="st")
                    nc.tensor.transpose(yn, y_sb[:, sl], ident[:11, :11])
                    p2 = tp2.tile([128, 1], F32, name=f"p2_{rt}", tag="st")
                    nc.tensor.transpose(p2, s2_sb[:, sl], ident[:1, :1])
                    mu_n = tail.tile([128, 1], F32, name=f"mu{rt}", tag="mu", bufs=2)
                    nc.scalar.mul(mu_n, yn[:, 10:11], -DT_STEP / D)   # -mean(h)
                    ex2 = tail.tile([128, 1], F32, name=f"ex2_{rt}", tag="ex2", bufs=2)
                    nc.scalar.mul(ex2, p2, DT_STEP * DT_STEP / D)     # E[h^2]
                    var = tail.tile([128, 1], F32, name=f"var{rt}", tag="var", bufs=2)
                    # var = E[h^2] - mu^2 = -(mu_n*mu_n) + ex2
                    nc.vector.scalar_tensor_tensor(var, in0=mu_n, scalar=-1.0,
                                                   op0=ALU.mult, in1=mu_n,
                                                   op1=ALU.mult)
                    nc.vector.tensor_add(var, var, ex2)
                    sd = tail.tile([128, 1], F32, name=f"sd{rt}", tag="sd", bufs=2)
                    nc.scalar.activation(sd, var, AF.Sqrt, bias=eps_sb, scale=1.0)
                    inv = tail.tile([128, 1], F32, name=f"inv{rt}", tag="inv", bufs=2)
                    nc.vector.reciprocal(inv, sd)
                    qn = tail.tile([128, 1], F32, name=f"qn{rt}", tag="qn", bufs=2)
                    nc.vector.tensor_mul(qn, mu_n, inv)               # -mu*inv

                    t1 = tail.tile([128, 10], F32, name=f"t1_{rt}", tag="t1", bufs=2)
                    nc.vector.tensor_scalar_mul(t1, yn[:, 0:10], inv)
                    t2 = tail.tile([128, 10], F32, name=f"t2_{rt}", tag="t2", bufs=2)
                    nc.vector.scalar_tensor_tensor(t2, in0=w1_bc, scalar=qn,
                                                   in1=t1, op0=ALU.mult, op1=ALU.add)
                    o = tail.tile([128, 10], F32, name=f"o{rt}", tag="o", bufs=2)
                    nc.vector.tensor_add(o, t2, b2_bc)
                    nc.sync.dma_start(out=out[sl, :], in_=o)
            tp2ctx.close()

    nc.compile()
    return nc


_NC_CACHE = None


def _get_program():
    global _NC_CACHE
    if _NC_CACHE is None:
        _NC_CACHE = _build_program()
    return _NC_CACHE


def _prepare_in_maps(inputs):
    x = np.asarray(inputs["x"], dtype=np.float32)
    w_enc = np.asarray(inputs["W_enc"], dtype=np.float32)
    w_res = np.asarray(inputs["W_res"], dtype=np.float32)
    w_in = np.asarray(inputs["W_in"], dtype=np.float32)
    bias = np.asarray(inputs["bias"], dtype=np.float32)
    ln_g = np.asarray(inputs["ln_g"], dtype=np.float32)
    ln_b = np.asarray(inputs["ln_b"], dtype=np.float32)
    w_out = np.asarray(inputs["W_out"], dtype=np.float32)
    b_out = np.asarray(inputs["b_out"], dtype=np.float32)

    w_c = (w_enc.T.astype(np.float64) @ w_in.astype(np.float64)).astype(np.float32)
    w2 = w_out * ln_g[None, :]                       # [10, D]
    # fp8 stationary weights in the S-scaled domain: [128, KD, D],
    # element (p, ks, m) = S*0.2*W_res[ks*128+p, m]
    w8 = (S * DT_STEP * w_res).astype(E4NP).reshape(KD, 128, D).transpose(1, 0, 2)
    w2a = np.empty((D, 11), np.float32)
    w2a[:, :10] = DT_STEP * w2.T                     # readout: gives W2 @ h.T
    w2a[:, 10] = 1.0                                 # S1 column: sum_D u
    w1v = w2.sum(axis=1).astype(np.float32)
    b2v = (w_out.astype(np.float64) @ ln_b.astype(np.float64)
           + b_out.astype(np.float64)).astype(np.float32)

    shared = {
        "wc16": np.ascontiguousarray(w_c.astype(BF16NP)),
        "w8": np.ascontiguousarray(w8),
        "bias": np.ascontiguousarray(bias),
        "w2a": np.ascontiguousarray(w2a),
        "w1": np.ascontiguousarray(w1v),
        "b2": np.ascontiguousarray(b2v),
    }
    in_maps = []
    for c in range(N_CORES):
        m = dict(shared)
        m["x16"] = np.ascontiguousarray(x[c * R:(c + 1) * R, :].astype(BF16NP))
        in_maps.append(m)
    return in_maps


def run(inputs, trace=False, tmpdir=None):
    """Run on 8 NeuronCores; returns (out [8192,10], BassKernelResults)."""
    nc = _get_program()
    in_maps = _prepare_in_maps(inputs)
    res = bass_utils.run_bass_kernel_spmd(
        nc, in_maps, core_ids=list(range(N_CORES)), trace=trace, tmpdir=tmpdir)
    outs = [np.asarray(r["out"]) for r in res.results]
    return np.concatenate(outs, axis=0), res


def kernel(**inputs):
    out, _ = run(inputs, trace=False)
    return out


# revision 24
# speedup vs baseline: 1.5425x; 1.1568x over previous
"""Trainium2 Bass kernel for nn_ChimeraNet (encoder -> 10-step Euler RNN -> LN -> readout).

Data-parallel over 8 NeuronCores: each core gets 1024 rows of the batch and a
replicated set of (host-prefolded) weights.

Math (per core, R=1024 rows, D=1024), with u = h/0.2 so the update is
    u_{t+1} = 0.8*u_t + tanh(u_t @ (0.2 W_res) + drive_in),   u_1 = tanh(drive_in)

fp8 fast path: the recurrent matmul runs in fp8-e4m3 DoubleRow mode (K=256 per
instruction at 0.5 cycles/row -> ~4x the fp32r rate).  Everything is kept in a
scaled PSUM domain: psum = S*(u @ 0.2*W_res + drive_in) with S=32, where
  - W8    = e4m3(S * 0.2 * W_res), stationary tiles [128, 8, D]
  - u8    = e4m3(u) cast each step from the fp16 state (DVE/gpsimd copies)
  - drive_in enters PSUM via a DoubleRow identity matmul on a hi/lo fp8 pair
    (dHi = e4m3(S*drive_in), dLo = e4m3(S*drive_in - dHi); error ~ (2%)^2)
The tanh then reads psum pairs [128,1024] with scale=1/S on the ACT engine and
writes v in fp16; the DVE keeps the real state u in fp16 (u' = 0.8u + v).
LayerNorm+readout are folded as in the f32r baseline but run in fp16 (exact
ones-column for S1, squares via DVE, stats chain in f32).

Encoder x @ (W_enc.T W_in) runs in bf16 (same PE rate as f32r, half the DMA).
"""

import os
import sys

import numpy as np
import ml_dtypes

try:
    import concourse.bass as bass  # noqa: F401
except ImportError:  # pragma: no cover - fresh grading env without PYTHONPATH
    for p in ("/root/.axon_site", "/root/.axon_site/_ro/trn_rl_repo",
              "/root/.axon_site/_ro/pypackages", "/opt/trn_rl_repo"):
        if os.path.isdir(p) and p not in sys.path:
            sys.path.append(p)
    import concourse.bass as bass

from contextlib import ExitStack

import concourse.tile as tile
from concourse import bacc, bass_utils, mybir
from concourse.masks import make_identity

N_CORES = 8
B = 8192
R = B // N_CORES        # rows per core
D = 1024                # latent dim
KX = 784                # encoder input dim
DT_STEP = 0.2
STEPS = 10
EPS = 1e-5
S = 32.0                # fp8 psum domain scale

F32 = mybir.dt.float32
BF16 = mybir.dt.bfloat16
FP16 = mybir.dt.float16
E4 = mybir.dt.float8e4
AF = mybir.ActivationFunctionType
ALU = mybir.AluOpType
DR = mybir.MatmulPerfMode.DoubleRow

KD = D // 128           # 8 k/m tiles over D
NS = R // 512           # 2 moving-dim slices of 512
KXT = [128] * 6 + [16]  # 784 = 6*128 + 16
NWARM = 6               # PE warmup matmuls (HAM un-throttle during DMA wait)

E4NP = ml_dtypes.float8_e4m3
BF16NP = ml_dtypes.bfloat16


def _build_program():
    nc = bacc.Bacc("TRN2", target_bir_lowering=False, debug=False)

    x16 = nc.dram_tensor("x16", [R, KX], BF16, kind="ExternalInput").ap()
    wc16 = nc.dram_tensor("wc16", [KX, D], BF16, kind="ExternalInput").ap()
    w8 = nc.dram_tensor("w8", [128, KD, D], E4, kind="ExternalInput").ap()
    bias = nc.dram_tensor("bias", [D], F32, kind="ExternalInput").ap()
    w2a = nc.dram_tensor("w2a", [D, 11], F32, kind="ExternalInput").ap()
    w1 = nc.dram_tensor("w1", [10], F32, kind="ExternalInput").ap()
    b2 = nc.dram_tensor("b2", [10], F32, kind="ExternalInput").ap()
    out = nc.dram_tensor("out", [R, 10], F32, kind="ExternalOutput").ap()

    with tile.TileContext(nc) as tc, ExitStack() as ctx:
        state = ctx.enter_context(tc.tile_pool(name="state", bufs=1))
        consts = ctx.enter_context(tc.tile_pool(name="consts", bufs=1))
        wres_pool = ctx.enter_context(tc.tile_pool(name="wres", bufs=1))

        # persistent SBUF state (all transposed: D on partitions, rows free).
        # u/v in f32: DVE ALU ops run at full rate in f32 (fp16 is half rate).
        # u tiles are f32r so the tail readout matmul can consume them directly.
        F32R = mybir.dt.float32r
        u_sb = [state.tile([128, KD, R], F32R, name=f"u{b}", tag=f"u{b}")
                for b in range(2)]
        u8_sb = [state.tile([128, KD, R], E4, name=f"u8{b}", tag=f"u8{b}")
                 for b in range(2)]
        # v holds a single 512-row n-slice: it is consumed by the stt
        # immediately after the tanh that produces it.
        v_sb = state.tile([128, KD, 512], F32, name="v", tag="v")
        sq_sb = state.tile([128, KD, R], FP16, name="sq", tag="sq")
        dinHL = state.tile([128, 2 * KD, R], E4, name="dinHL", tag="dinHL")
        w8_sb = wres_pool.tile([128, KD, D], E4, name="w8sb", tag="w8sb")

        with ExitStack() as mmctx:
            # PE warmup: dependency-free fp32 matmuls starting at t~0 pull
            # the HAM clock gate to 8/8 while the input DMAs are in flight.
            warmctx = ExitStack()
            warm_psum = warmctx.enter_context(
                tc.tile_pool(name="warm", bufs=1, space="PSUM"))
            warm_src = consts.tile([128, 256], F32)
            nc.vector.memset(warm_src, 0.01)
            warm_sb = consts.tile([128, 1], F32)
            for w in range(NWARM):
                wp = warm_psum.tile([128, 512], F32, name=f"warm{w}", tag="wm")
                nc.tensor.matmul(wp[:, :256], lhsT=warm_src[:, :128], rhs=warm_src,
                                 start=True, stop=True)
                if w == NWARM - 1:
                    nc.vector.tensor_copy(warm_sb, wp[:, :1])  # keep-alive

            ident = consts.tile([128, 128], F32)
            make_identity(nc, ident)
            identb = consts.tile([128, 128], BF16)
            nc.vector.tensor_copy(identb, ident)
            idpair = consts.tile([128, 2, 128], E4)
            nc.vector.tensor_copy(idpair[:, 0, :], ident)
            nc.vector.tensor_copy(idpair[:, 1, :], ident)
            bias_sb = consts.tile([128, KD], F32)
            nc.gpsimd.dma_start(out=bias_sb, in_=bias.rearrange("(m p) -> p m", p=128))

            # W8 arrives on the gpsimd queue while x streams on sync.
            nc.gpsimd.dma_start(out=w8_sb, in_=w8)

            # ------------ encoder: x -> x.T (bf16), drive = x @ W_c + bias ----
            # drive tiles rotate through a small pool: each (m,n) tile is dead
            # as soon as its tanh + dHi/dLo consumers have run.
            drive_pool = ExitStack()
            dp = drive_pool.enter_context(tc.tile_pool(name="drv", bufs=4))

            with ExitStack() as enc:
                xn_pool = enc.enter_context(tc.tile_pool(name="xn", bufs=4))
                xt_pool = enc.enter_context(tc.tile_pool(name="xt", bufs=1))
                wc_pool = enc.enter_context(tc.tile_pool(name="wc", bufs=1))
                etp = enc.enter_context(
                    tc.tile_pool(name="etp", bufs=3, space="PSUM"))
                eps_pool = enc.enter_context(
                    tc.tile_pool(name="emm", bufs=4, space="PSUM"))

                xt_big = xt_pool.tile([128, len(KXT), R], BF16, name="xt_big")
                wc_sb = [wc_pool.tile([128, D], BF16, name=f"wc{k}", tag=f"wc{k}")
                         for k in range(len(KXT))]
                for k, kw in enumerate(KXT):
                    nc.scalar.dma_start(out=wc_sb[k][:kw, :],
                                        in_=wc16[k * 128:k * 128 + kw, :])

                def transpose_rt(rt):
                    xn = xn_pool.tile([128, KX], BF16, name=f"xn{rt}", tag="xn")
                    nc.sync.dma_start(out=xn, in_=x16[rt * 128:(rt + 1) * 128, :])
                    rsl = slice(rt * 128, (rt + 1) * 128)
                    # dependency-free matmul BEFORE the transposes: runs during
                    # this tile's DMA wait, keeps the HAM window at 8/8
                    wp0 = warm_psum.tile([128, 512], F32, name=f"wmh{rt}", tag="wm")
                    nc.tensor.matmul(wp0[:, :256], lhsT=warm_src[:, :128],
                                     rhs=warm_src, start=True, stop=True)
                    for kp in range(3):
                        pt = etp.tile([128, 256], BF16, name=f"pt{rt}_{kp}", tag="tp")
                        for h in range(2):
                            k = 2 * kp + h
                            nc.tensor.transpose(pt[:, h * 128:(h + 1) * 128],
                                                xn[:, k * 128:(k + 1) * 128], identb)
                        src = pt.rearrange("p (two c) -> p two c", two=2)
                        dst = xt_big[:, 2 * kp:2 * kp + 2, rsl]
                        if kp % 2 == 0:
                            nc.scalar.copy(dst, src)
                        else:
                            nc.vector.tensor_copy(dst, src)
                    pt = etp.tile([128, 256], BF16, name=f"pt{rt}_3", tag="tp")
                    nc.tensor.transpose(pt[:16, :128], xn[:, 768:784], identb)
                    nc.vector.tensor_copy(xt_big[:16, 6, rsl], pt[:16, :128])

                def encoder_mms(n):
                    sl = slice(n * 512, (n + 1) * 512)
                    for mp in range(KD // 2):
                        # paired drive tile [128, 2, 512] -> wide consumer ops
                        dpair = dp.tile([128, 2, 512], F32, name=f"dr{n}_{mp}",
                                        tag="dr")
                        for half in range(2):
                            m = 2 * mp + half
                            ps = eps_pool.tile([128, 512], F32,
                                               name=f"eps{n}_{m}", tag="emm")
                            for k, kw in enumerate(KXT):
                                nc.tensor.matmul(
                                    ps,
                                    lhsT=wc_sb[k][:kw, m * 128:(m + 1) * 128],
                                    rhs=xt_big[:kw, k, sl],
                                    start=(k == 0), stop=(k == len(KXT) - 1))
                            nc.scalar.activation(dpair[:, half, :], ps,
                                                 AF.Identity,
                                                 bias=bias_sb[:, m:m + 1],
                                                 scale=1.0)
                        mm2 = slice(2 * mp, 2 * mp + 2)
                        # u_1 = v_0 = tanh(drive_in)  (f32r state), pair-wide
                        nc.scalar.activation(u_sb[1][:, mm2, sl], dpair, AF.Tanh)
                        # dHi = e4m3(S*drive) = 33*drive - drive; dLo = S*drive - dHi
                        dstH = dinHL.rearrange("p (m two) r -> p m two r", two=2)
                        nc.vector.scalar_tensor_tensor(
                            dstH[:, mm2, 0, sl], in0=dpair, scalar=S + 1.0,
                            op0=ALU.mult, in1=dpair, op1=ALU.subtract)
                        nc.vector.scalar_tensor_tensor(
                            dstH[:, mm2, 1, sl], in0=dpair, scalar=S,
                            op0=ALU.mult, in1=dstH[:, mm2, 0, sl],
                            op1=ALU.subtract)
                        nc.vector.tensor_copy(u8_sb[1][:, mm2, sl],
                                              u_sb[1][:, mm2, sl])

                # interleave: the n=0 encoder matmuls run while rows 4-7 DMA in
                for rt in range(4):
                    transpose_rt(rt)
                encoder_mms(0)
                for rt in range(4, 8):
                    transpose_rt(rt)
                encoder_mms(1)
            drive_pool.close()
            warmctx.close()

            # ------------ Euler integration loop (fp8 DoubleRow) --------------
            loopctx = ExitStack()
            psum = loopctx.enter_context(
                tc.tile_pool(name="mm", bufs=3, space="PSUM"))

            for s in range(1, STEPS):
                cur, nxt = s % 2, (s + 1) % 2
                for n in range(NS):
                    sl = slice(n * 512, (n + 1) * 512)
                    for mp in range(KD // 2):
                        ps = psum.tile([128, 1024], F32, name=f"ps{s}_{n}_{mp}",
                                       tag="mm")
                        for half in range(2):
                            m = 2 * mp + half
                            psh = ps[:, half * 512:(half + 1) * 512]
                            nc.tensor.matmul(psh, lhsT=idpair,
                                             rhs=dinHL[:, 2 * m:2 * m + 2, sl],
                                             start=True, stop=False, perf_mode=DR)
                            for k2 in range(KD // 2):
                                nc.tensor.matmul(
                                    psh,
                                    lhsT=w8_sb[:, 2 * k2:2 * k2 + 2,
                                               m * 128:(m + 1) * 128],
                                    rhs=u8_sb[cur][:, 2 * k2:2 * k2 + 2, sl],
                                    start=False, stop=(k2 == KD // 2 - 1),
                                    perf_mode=DR)
                        mm2 = slice(2 * mp, 2 * mp + 2)
                        # v = tanh(psum/S) for both halves in one ACT op
                        nc.scalar.activation(
                            v_sb[:, mm2, :],
                            ps.rearrange("p (two c) -> p two c", two=2),
                            AF.Tanh, scale=1.0 / S)
                        # pair-wide state update on DVE (~690ns for 1024 elems)
                        nc.vector.scalar_tensor_tensor(
                            u_sb[nxt][:, mm2, sl], in0=u_sb[cur][:, mm2, sl],
                            scalar=1.0 - DT_STEP, op0=ALU.mult,
                            in1=v_sb[:, mm2, :], op1=ALU.add)
                        if s < STEPS - 1:
                            # e4m3 cast for the next step's matmuls; alternate
                            # DVE / ACT to keep both under the PE step time
                            if mp % 2 == 0:
                                nc.vector.tensor_copy(u8_sb[nxt][:, mm2, sl],
                                                      u_sb[nxt][:, mm2, sl])
                            else:
                                nc.scalar.copy(u8_sb[nxt][:, mm2, sl],
                                               u_sb[nxt][:, mm2, sl])
                        else:
                            # last step: u8 is dead; compute LN squares instead
                            if mp % 2 == 0:
                                nc.vector.tensor_mul(sq_sb[:, mm2, sl],
                                                     u_sb[nxt][:, mm2, sl],
                                                     u_sb[nxt][:, mm2, sl])
                            else:
                                nc.scalar.square(sq_sb[:, mm2, sl],
                                                 u_sb[nxt][:, mm2, sl])

            loopctx.close()
            uf = u_sb[STEPS % 2]

            # ------------ tail: LN stats + readout (fp16 matmuls) -------------
            tail = ctx.enter_context(tc.tile_pool(name="tail", bufs=1))
            tmm = mmctx.enter_context(
                tc.tile_pool(name="tmm", bufs=2, space="PSUM"))

            ones_f32 = tail.tile([128, 1], F32)
            nc.vector.memset(ones_f32, 1.0)
            ones_sb = tail.tile([128, 1], FP16)
            nc.scalar.copy(ones_sb, ones_f32)
            eps_sb = tail.tile([128, 1], F32)
            nc.vector.memset(eps_sb, EPS)
            # w2a = [0.2*W2.T | ones] : readout weights + S1 column
            w2a_sb = tail.tile([128, KD, 11], F32R)
            nc.gpsimd.dma_start(out=w2a_sb,
                                in_=w2a.rearrange("(k p) o -> p k o",
                                                  p=128).bitcast(F32R))
            w1_bc = tail.tile([128, 10], F32)
            nc.gpsimd.dma_start(out=w1_bc, in_=bass.AP(tensor=w1.tensor, offset=w1.offset,
                                                       ap=[[0, 128]] + list(w1.ap)))
            b2_bc = tail.tile([128, 10], F32)
            nc.gpsimd.dma_start(out=b2_bc, in_=bass.AP(tensor=b2.tensor, offset=b2.offset,
                                                       ap=[[0, 128]] + list(b2.ap)))

            s2_sb = tail.tile([1, R], F32)
            y_sb = tail.tile([11, R], F32)

            # per-n readout matmuls, with the per-row-tile stat/combine chains
            # interleaved so the n=0 half finishes while n=1 runs.
            # y_sb rows 0-9 = 0.2*(W2 @ u.T) = W2 @ h.T;  row 10 = sum_D u.
            tp2ctx = ExitStack()
            tp2 = tp2ctx.enter_context(
                tc.tile_pool(name="tp2", bufs=4, space="PSUM"))
            for n in range(NS):
                sl = slice(n * 512, (n + 1) * 512)
                yp = tmm.tile([128, 512], F32, name=f"yp{n}", tag="tmm")
                for k in range(KD):
                    nc.tensor.matmul(yp[:11, :], lhsT=w2a_sb[:, k, :],
                                     rhs=uf[:, k, sl],
                                     start=(k == 0), stop=(k == KD - 1))
                nc.scalar.copy(y_sb[:, sl], yp[:11, :])
                s2 = tmm.tile([128, 512], F32, name=f"s2p{n}", tag="tmm")
                for k in range(KD):
                    nc.tensor.matmul(s2[:1, :], lhsT=ones_sb,
                                     rhs=sq_sb[:, k, sl],
                                     start=(k == 0), stop=(k == KD - 1))
                nc.scalar.copy(s2_sb[:, sl], s2[:1, :])

                for rt in range(n * 4, (n + 1) * 4):
                    sl = slice(rt * 128, (rt + 1) * 128)
                    yn = tp2.tile([128, 11], F32, name=f"yn{rt}", tag="st")
                    nc.tensor.transpose(yn, y_sb[:, sl], ident[:11, :11])
                    p2 = tp2.tile([128, 1], F32, name=f"p2_{rt}", tag="st")
                    nc.tensor.transpose(p2, s2_sb[:, sl], ident[:1, :1])
                    mu_n = tail.tile([128, 1], F32, name=f"mu{rt}", tag="mu", bufs=2)
                    nc.scalar.mul(mu_n, yn[:, 10:11], -DT_STEP / D)   # -mean(h)
                    ex2 = tail.tile([128, 1], F32, name=f"ex2_{rt}", tag="ex2", bufs=2)
                    nc.scalar.mul(ex2, p2, DT_STEP * DT_STEP / D)     # E[h^2]
                    var = tail.tile([128, 1], F32, name=f"var{rt}", tag="var", bufs=2)
                    # var = E[h^2] - mu^2 = -(mu_n*mu_n) + ex2
                    nc.vector.scalar_tensor_tensor(var, in0=mu_n, scalar=-1.0,
                                                   op0=ALU.mult, in1=mu_n,
                                                   op1=ALU.mult)
                    nc.vector.tensor_add(var, var, ex2)
                    sd = tail.tile([128, 1], F32, name=f"sd{rt}", tag="sd", bufs=2)
                    nc.scalar.activation(sd, var, AF.Sqrt, bias=eps_sb, scale=1.0)
                    inv = tail.tile([128, 1], F32, name=f"inv{rt}", tag="inv", bufs=2)
                    nc.vector.reciprocal(inv, sd)
                    qn = tail.tile([128, 1], F32, name=f"qn{rt}", tag="qn", bufs=2)
                    nc.vector.tensor_mul(qn, mu_n, inv)               # -mu*inv

                    t1 = tail.tile([128, 10], F32, name=f"t1_{rt}", tag="t1", bufs=2)
                    nc.vector.tensor_scalar_mul(t1, yn[:, 0:10], inv)
                    t2 = tail.tile([128, 10], F32, name=f"t2_{rt}", tag="t2", bufs=2)
                    nc.vector.scalar_tensor_tensor(t2, in0=w1_bc, scalar=qn,
                                                   in1=t1, op0=ALU.mult, op1=ALU.add)
                    o = tail.tile([128, 10], F32, name=f"o{rt}", tag="o", bufs=2)
                    nc.vector.tensor_add(o, t2, b2_bc)
                    nc.sync.dma_start(out=out[sl, :], in_=o)
            tp2ctx.close()

    nc.compile()
    return nc


_NC_CACHE = None


def _get_program():
    global _NC_CACHE
    if _NC_CACHE is None:
        _NC_CACHE = _build_program()
    return _NC_CACHE


def _prepare_in_maps(inputs):
    x = np.asarray(inputs["x"], dtype=np.float32)
    w_enc = np.asarray(inputs["W_enc"], dtype=np.float32)
    w_res = np.asarray(inputs["W_res"], dtype=np.float32)
    w_in = np.asarray(inputs["W_in"], dtype=np.float32)
    bias = np.asarray(inputs["bias"], dtype=np.float32)
    ln_g = np.asarray(inputs["ln_g"], dtype=np.float32)
    ln_b = np.asarray(inputs["ln_b"], dtype=np.float32)
    w_out = np.asarray(inputs["W_out"], dtype=np.float32)
    b_out = np.asarray(inputs["b_out"], dtype=np.float32)

    w_c = (w_enc.T.astype(np.float64) @ w_in.astype(np.float64)).astype(np.float32)
    w2 = w_out * ln_g[None, :]                       # [10, D]
    # fp8 stationary weights in the S-scaled domain: [128, KD, D],
    # element (p, ks, m) = S*0.2*W_res[ks*128+p, m]
    w8 = (S * DT_STEP * w_res).astype(E4NP).reshape(KD, 128, D).transpose(1, 0, 2)
    w2a = np.empty((D, 11), np.float32)
    w2a[:, :10] = DT_STEP * w2.T                     # readout: gives W2 @ h.T
    w2a[:, 10] = 1.0                                 # S1 column: sum_D u
    w1v = w2.sum(axis=1).astype(np.float32)
    b2v = (w_out.astype(np.float64) @ ln_b.astype(np.float64)
           + b_out.astype(np.float64)).astype(np.float32)

    shared = {
        "wc16": np.ascontiguousarray(w_c.astype(BF16NP)),
        "w8": np.ascontiguousarray(w8),
        "bias": np.ascontiguousarray(bias),
        "w2a": np.ascontiguousarray(w2a),
        "w1": np.ascontiguousarray(w1v),
        "b2": np.ascontiguousarray(b2v),
    }
    in_maps = []
    for c in range(N_CORES):
        m = dict(shared)
        m["x16"] = np.ascontiguousarray(x[c * R:(c + 1) * R, :].astype(BF16NP))
        in_maps.append(m)
    return in_maps


def run(inputs, trace=False, tmpdir=None):
    """Run on 8 NeuronCores; returns (out [8192,10], BassKernelResults)."""
    nc = _get_program()
    in_maps = _prepare_in_maps(inputs)
    res = bass_utils.run_bass_kernel_spmd(
        nc, in_maps, core_ids=list(range(N_CORES)), trace=trace, tmpdir=tmpdir)
    outs = [np.asarray(r["out"]) for r in res.results]
    return np.concatenate(outs, axis=0), res


def kernel(**inputs):
    out, _ = run(inputs, trace=False)
    return out
